# revision 22
# baseline (speedup 1.0000x reference)
"""Trainium2 Bass kernel for the supervoxel erode/edge loss module.

Math: the reference pads the [B,X,Y] grid by (4sx, 4sy), tiles it into 8x8
patches, zeroes each patch's last row/col of the mask channel, erodes along
both patch axes (`a*b + (1-a)*a + (1-b)*a` == `2a - a^2` with a the
neighbor product) and sums eroded*edge over all cells, then takes the mean
over (B, patches).  In padded coords u = x + 4sx, v = y + 4sy the patch
grid is [0,1032)^2; a cell contributes iff u%8 <= 5 and v%8 <= 5, and its
contribution uses only raw mask values:

    ax = m(u,v)*m(u+1,v); ay = m(u,v)*m(u,v+1)
    term = ax*(2-ax) * ay*(2-ay) * e(u,v)

so the loss decomposes into independent 7-row blocks (u in [8k, 8k+6]) x
7-col groups (v in [8g, 8g+6]) with a 6x6 live base grid per block/group.

With Q(a) = (a-1)^2 (so a(2-a) = 1-Q(a)):

    term = (1 - Qx)(1 - Qy) e = z - w,  w = (Qy - 1) e,  z = Qx w

Host staging (pure slicing + zero-fill, no arithmetic on values), all bf16:
    maskb{L,R} [NB, 7*hg*7]  -- mask channel col-halves, zero outside image
    edgeb{L,R} [NB, 6*hg*6]  -- edge at base cells
    runtb      [128, 4*W]    -- leftover (<128) blocks' vb/vr/vc/e cells
                                gathered dense across all 128 partitions
NB = B * (#row blocks); cores take contiguous block ranges (data-parallel;
the mean is one scalar so the combine happens on host, no collective).

Device, per subunit (tile x column-half; p = block), 3-stage software
pipeline with the runt first (it has the smallest DMA so it fills the
head while the first mask tile streams in):

    A (DVE):  ay0 = vb*vc ; ax0 = vb*vr          (bf16 muls run 2x)
    B (Act):  sqy = Square(ay0 - 1) ; sqx = Square(ax0 - 1)
    C (DVE):  w = (sqy - 1)*e  [accum Sw] ; z = sqx*w  [accum Sz]

host total = sum(Sz) - sum(Sw).  DVE is the critical engine at ~4 passes
per element; Act's squares hide under the neighbouring subunits' A/C
stages.  (Custom DVE ops and native tensor_tensor_reduce would cut this
further but crash this runtime; engines contend for SBUF so Pool/Act
offload beyond this is net-negative.)

DMA: everything issued up front on the gpsimd software-DGE queue (the
hardware-DGE queues cap at ~25-37 GB/s; SWDGE hits ~344 GB/s) as one
6-12 KiB descriptor per (block, input, half): ~1030 packets/core vs the
naive layout's ~5500 -- packet cadence, not bandwidth, is the real limit.
"""

import sys

sys.path.insert(0, "/opt/trn_rl_repo")

import numpy as np

from concourse import bacc, mybir, tile
from concourse.bass_utils import run_bass_kernel_spmd

F32 = mybir.dt.float32
BF16 = mybir.dt.bfloat16
N_CORES = 8
SHIFTS = [(0, 0), (1, 0), (0, 1), (1, 1)]
DX = 8


def _build_program(n_full, NG, W_runt, niter=1):
    """Per-core program. Inputs (bf16): maskbL/R [NB, 7*hg*7], edgebL/R
    [NB, 6*hg*6], runtb [128, 4*W_runt]. Output: out [128, U] f32 with
    (Sw, Sz) column pairs per subunit."""
    NB = n_full * 128
    h = NG // 2
    HGS = [h, NG - h]
    NSU = 2 * n_full + (1 if W_runt else 0)
    U = 2 * NSU

    nc = bacc.Bacc("TRN2", target_bir_lowering=False, debug=False)
    mh = [
        nc.dram_tensor(f"maskb{s}", [NB, 7 * hg * 7], BF16, kind="ExternalInput")
        for s, hg in zip("LR", HGS)
    ]
    eh = [
        nc.dram_tensor(f"edgeb{s}", [NB, 6 * hg * 6], BF16, kind="ExternalInput")
        for s, hg in zip("LR", HGS)
    ]
    if W_runt:
        runtb_h = nc.dram_tensor("runtb", [128, 4 * W_runt], BF16, kind="ExternalInput")
    out_h = nc.dram_tensor("out", [128, U], F32, kind="ExternalOutput")

    with tile.TileContext(nc) as tc:
        with (
            tc.tile_pool(name="mt", bufs=2 * n_full) as mt_pool,
            tc.tile_pool(name="et", bufs=2 * n_full) as et_pool,
            tc.tile_pool(name="wa", bufs=3) as wa_pool,
            tc.tile_pool(name="ws", bufs=3) as ws_pool,
            tc.tile_pool(name="wz", bufs=2) as wz_pool,
            tc.tile_pool(name="c", bufs=1) as c_pool,
        ):
            partials = c_pool.tile([128, U], F32)
            bm1 = c_pool.tile([128, 1], F32, name="bm1")
            rt = c_pool.tile([128, 4 * W_runt], BF16, name="rt") if W_runt else None

            def emit_iter():
                nc.vector.memset(bm1[:], -1.0)
                # runt last: the tail then ends on its tiny C-stage
                subs = [(u, s) for u in range(n_full) for s in (0, 1)] + (
                    [(-1, 0)] if W_runt else []
                )
                if W_runt:
                    nc.gpsimd.dma_start(rt[:], runtb_h.ap())
                tiles = {}
                for u in range(n_full):
                    for s in (0, 1):
                        hg = HGS[s]
                        mt = mt_pool.tile([128, 7 * hg * 7], BF16, name="mt")
                        et = et_pool.tile([128, 6 * hg * 6], BF16, name="et")
                        nc.gpsimd.dma_start(
                            mt[:], mh[s].ap()[u * 128 : (u + 1) * 128, :]
                        )
                        nc.gpsimd.dma_start(
                            et[:], eh[s].ap()[u * 128 : (u + 1) * 128, :]
                        )
                        tiles[(u, s)] = (mt, et)

                sq = mybir.ActivationFunctionType.Square
                st = {}

                def sub_views(key):
                    u, s = key
                    if u >= 0:
                        hg = HGS[s]
                        mt, et = tiles[key]
                        m4 = mt[:].rearrange("p (r g j) -> p r g j", r=7, j=7)
                        n = 6 * hg * 6
                        return (
                            m4[:, 0:6, :, 0:6], m4[:, 1:7, :, 0:6],
                            m4[:, 0:6, :, 1:7], et[:], n, True,
                        )
                    W = W_runt
                    return (
                        rt[:, 0:W], rt[:, W : 2 * W], rt[:, 2 * W : 3 * W],
                        rt[:, 3 * W : 4 * W], W, False,
                    )

                def swv(t, full):
                    return (t[:].rearrange("p (r g j) -> p r g j", r=6, j=6)
                            if full else t[:])

                def stage_a(i):
                    vb, vr, vc, ev, n, full = sub_views(subs[i])
                    ay0 = wa_pool.tile([128, n], BF16, name="ay0")
                    ax0 = wa_pool.tile([128, n], BF16, name="ax0")
                    nc.vector.tensor_mul(swv(ay0, full), vb, vc)
                    nc.vector.tensor_mul(swv(ax0, full), vb, vr)
                    st[i] = (ax0, ay0, ev, n)

                def stage_b(i):
                    ax0, ay0, ev, n = st[i]
                    sqy = ws_pool.tile([128, n], BF16, name="sqy")
                    sqx = ws_pool.tile([128, n], BF16, name="sqx")
                    nc.scalar.activation(sqy[:], ay0[:], sq, bias=bm1[:])
                    nc.scalar.activation(sqx[:], ax0[:], sq, bias=bm1[:])
                    st[i] = (sqx, sqy, ev, n)

                zs = {}

                def stage_c(i):
                    sqx, sqy, ev, n = st.pop(i)
                    w = wz_pool.tile([128, n], BF16, name="w")
                    z = wz_pool.tile([128, n], BF16, name="z")
                    nc.vector.scalar_tensor_tensor(
                        w[:], sqy[:], 1.0, ev,
                        op0=mybir.AluOpType.subtract, op1=mybir.AluOpType.mult,
                        accum_out=partials[:, 2 * i : 2 * i + 1],
                    )
                    # z = sqx*w as a (2x-rate) mul; its reduction rides Act
                    nc.vector.tensor_mul(z[:], sqx[:], w[:])
                    zs[i] = (z, n)

                def stage_cp(i):
                    z, n = zs.pop(i)
                    zj = wz_pool.tile([128, n], BF16, name="zj")
                    nc.scalar.activation(
                        zj[:], z[:], mybir.ActivationFunctionType.Copy,
                        accum_out=partials[:, 2 * i + 1 : 2 * i + 2],
                    )

                # per-engine queues: DVE = A0 A1 C0 A2 C1 ...;
                # Act = B0 B1 CP0 B2 CP1 ... -- each op's inputs are ready
                # one slot ahead, so neither engine stalls mid-pipeline
                stage_a(0)
                stage_b(0)
                for i in range(1, NSU):
                    stage_a(i)
                    stage_b(i)
                    stage_c(i - 1)
                    stage_cp(i - 1)
                stage_c(NSU - 1)
                stage_cp(NSU - 1)
                nc.sync.dma_start(out_h.ap(), partials[:])

            if niter == 1:
                emit_iter()
            else:
                with tc.For_i(0, niter, 1):
                    emit_iter()

    nc.compile()
    return nc


def _stage(mask, edge, idx):
    """Host-side slicing: maskb [B*KXb, 7, NG, 7] and edgeb [B*KXb, 6, NG, 6]
    f32 (dense; mask zero outside the image)."""
    B, X, Y = mask.shape
    sx, sy = SHIFTS[idx]
    KX = (X + DX) // DX
    KY = (Y + DX) // DX

    gy = np.arange(KY)
    y0 = 8 * gy[:, None] + np.arange(7)[None, :] - 4 * sy  # [KY, 7]
    g_ok = (y0 >= 0) & (y0 < Y)
    gsel = np.nonzero(g_ok.any(axis=1))[0]
    NG = len(gsel)
    ym = y0[gsel]
    yv = g_ok[gsel]
    kxs = np.arange(KX)
    x0 = 8 * kxs[:, None] + np.arange(7)[None, :] - 4 * sx  # [KX, 7]
    k_ok = (x0 >= 0) & (x0 < X)
    ksel = np.nonzero(k_ok.any(axis=1))[0]
    KXb = len(ksel)
    xm = x0[ksel]
    xv = k_ok[ksel]

    mc = mask[:, np.clip(xm.ravel(), 0, X - 1), :][:, :, np.clip(ym.ravel(), 0, Y - 1)]
    mc = mc.reshape(B, KXb * 7, NG * 7)
    vmask = (xv.ravel()[:, None] & yv.ravel()[None, :]).astype(mask.dtype)
    mc *= vmask
    maskb = mc.reshape(B * KXb, 7, NG, 7)

    # edge at base cells only: rows r=0..5, cols j=0..5 of each group
    xe = np.clip(xm[:, 0:6].ravel(), 0, X - 1)
    ye = np.clip(ym[:, 0:6].ravel(), 0, Y - 1)
    edgeb = edge[:, xe, :][:, :, ye].reshape(B * KXb, 6, NG, 6)

    norm = B * KX * KY
    return maskb, edgeb, NG, KXb, norm


def _stage_runt(maskb4, edgeb4, sel):
    """Gather leftover blocks' vb/vr/vc/e cells into [128, 4*W] (W padded)."""
    m4 = maskb4[sel]
    e4 = edgeb4[sel]
    vb = m4[:, 0:6, :, 0:6].ravel()
    vr = m4[:, 1:7, :, 0:6].ravel()
    vc = m4[:, 0:6, :, 1:7].ravel()
    ee = e4.ravel()
    n = vb.size
    W = -(-n // 128)
    out = np.zeros((4, 128 * W), dtype=maskb4.dtype)
    for i, a in enumerate((vb, vr, vc, ee)):
        out[i, :n] = a
    return np.ascontiguousarray(
        out.reshape(4, 128, W).transpose(1, 0, 2).reshape(128, 4 * W)
    ), W


def _run(mask, edge, loss_old, idx, trace=False, niter=1):
    import ml_dtypes

    B, X, Y, _ = mask.shape
    assert B % N_CORES == 0
    m3 = np.ascontiguousarray(mask[..., idx], dtype=np.float32)
    e3 = np.ascontiguousarray(edge[..., 0], dtype=np.float32)
    maskb, edgeb, NG, KXb, norm = _stage(m3, e3, idx)

    NBtot = maskb.shape[0]
    assert NBtot % N_CORES == 0
    NBc = NBtot // N_CORES
    n_full = NBc // 128
    runt_sel0 = np.arange(n_full * 128, NBc)
    h = NG // 2

    maskb4 = maskb.astype(ml_dtypes.bfloat16)
    edgeb4 = edgeb.astype(ml_dtypes.bfloat16)
    mhalves = [
        np.ascontiguousarray(maskb4[:, :, 0:h, :]).reshape(NBtot, -1),
        np.ascontiguousarray(maskb4[:, :, h:NG, :]).reshape(NBtot, -1),
    ]
    ehalves = [
        np.ascontiguousarray(edgeb4[:, :, 0:h, :]).reshape(NBtot, -1),
        np.ascontiguousarray(edgeb4[:, :, h:NG, :]).reshape(NBtot, -1),
    ]

    in_maps = []
    W_runt = 0
    for i in range(N_CORES):
        lo = i * NBc
        im = {}
        for s, tag in enumerate("LR"):
            im[f"maskb{tag}"] = mhalves[s][lo : lo + n_full * 128]
            im[f"edgeb{tag}"] = ehalves[s][lo : lo + n_full * 128]
        if len(runt_sel0):
            rb, W_runt = _stage_runt(maskb4, edgeb4, lo + runt_sel0)
            im["runtb"] = rb
        in_maps.append(im)

    nc = _build_program(n_full, NG, W_runt, niter=niter)
    res = run_bass_kernel_spmd(nc, in_maps, list(range(N_CORES)), trace=trace)
    total = 0.0
    for i in range(N_CORES):
        o = np.asarray(res.results[i]["out"], np.float64)
        total += o[:, 1::2].sum() - o[:, 0::2].sum()
    out = np.float32(np.asarray(loss_old, dtype=np.float32) + total / norm)
    return np.asarray(out, dtype=np.float32), res


def kernel(resized_image, mask_combined, edge_map, loss_old, mask_index):
    mask = np.asarray(mask_combined, dtype=np.float32)
    edge = np.asarray(edge_map, dtype=np.float32)
    idx = int(np.asarray(mask_index))
    out, _ = _run(mask, edge, loss_old, idx)
    return out
